# revision 37
# baseline (speedup 1.0000x reference)
"""Cantor global attention kernel for Trainium2 (8 NeuronCores, SPMD).

Strategy: data-parallel over batch B=64 -> 8 cores x 8 rows each.
All device tensors are 16-bit: Q/K (and the pre-exp score t) in fp16
for exponent accuracy, everything after the exp in bf16 for range
safety (scores reach ~|20| so e^t needs bf16's exponent range).  The
host uploads inputs already transposed into the SBUF layout
[proj][128 part][e*256 col] so every DMA is a few large contiguous
descriptors, and converts the bf16 output back to f32.

Per core, partition = b*16 + p//256; each expert owns 256 columns;
the W=3 neighbor gather becomes column offsets baked from the runtime
routes (slot-permuted so slot0 = self).

Engine placement (per core):
  - projection averaging (Q,K,V): DMA-accumulate (CCE add) - free
  - t_w = Qs*Ks_route:  DVE tensor_mul fp16 (2x packed), run-batched
  - gate: t *= sigmoid(beta) per non-self (e,w): DVE tensor_scalar
    (4x mode), immediate baked from betas
  - e_w = exp(esc*t):   ScalarE activation, uniform scale immediate
    esc = 0.25/(sqrt(128)*|temp|)  ->  fully batched big instructions
  - prod_w = e_w*Vs:    DVE tensor_mul bf16, run-batched
  - den|num = sum_w:    2 DVE adds per group over a [k=2,w=3] strided
    view covering both reductions
  - r = 0.5/den:        ScalarE ln (fp32 out) then exp(-x+ln 0.5)
  - out = num*r:        DVE mul, stored bf16
"""

import math

import numpy as np

import concourse.bass as bass
import concourse.mybir as mybir
from concourse import bacc, tile
from concourse.bass_utils import run_bass_kernel_spmd

E, NPROJ, B, P = 16, 2, 64, 4096
W = 3
EXPERT_DIM = 128
NCORES = 8
BS = B // NCORES          # 8 batch rows per core
COLS = 256                # free-dim columns per expert slab
PH = P // COLS            # 16 partition sub-blocks per batch row
PART = BS * PH            # 128 SBUF partitions
EC = E * COLS             # 4096 cols per w-block
GROUP = 4                 # experts per compute group
NG = E // GROUP           # 4 groups
GC = GROUP * COLS         # 1024 cols per group
ACT_SET_LN_EXP = 6        # act_info.json natural_log_exp_and_others

F16 = mybir.dt.float16
BF16 = mybir.dt.bfloat16
F32 = mybir.dt.float32
EXPF = mybir.ActivationFunctionType.Exp
LNF = mybir.ActivationFunctionType.Ln
MULT = mybir.AluOpType.mult
ADD = mybir.AluOpType.add


def _runs(pairs):
    """Split [(e, j), ...] into maximal runs of consecutive e and j."""
    runs = []
    for e, j in pairs:
        if runs and runs[-1][0] + runs[-1][2] == e and runs[-1][1] + runs[-1][2] == j:
            runs[-1][2] += 1
        else:
            runs.append([e, j, 1])
    return runs


def _build_nc(routes_s: np.ndarray, gates_s: np.ndarray, esc: float):
    nc = bacc.Bacc("TRN2", target_bir_lowering=False, debug=False,
                   num_devices=NCORES)

    q_d = nc.dram_tensor("q", [NPROJ, PART, EC], F16, kind="ExternalInput")
    k_d = nc.dram_tensor("k", [NPROJ, PART, EC], F16, kind="ExternalInput")
    v_d = nc.dram_tensor("v", [NPROJ, PART, EC], F16, kind="ExternalInput")
    o_d = nc.dram_tensor("out", [PART, EC], BF16, kind="ExternalOutput")

    def runs_for(e_lo, e_hi):
        out = []
        for w in range(W):
            pairs = [(e, int(routes_s[e, w])) for e in range(e_lo, e_hi)]
            for e0, j0, L in _runs(pairs):
                out.append((w, e0, j0, L))
        return out

    with tile.TileContext(nc) as tc:
        with (
            tc.tile_pool(name="io", bufs=1) as io_p,
            tc.tile_pool(name="mid", bufs=1) as mid_p,
        ):
            qs = io_p.tile([PART, EC], F16, name="qs", tag="qs")
            ks = io_p.tile([PART, EC], F16, name="ks", tag="ks")
            vs = io_p.tile([PART, EC], F16, name="vs", tag="vs")
            raws = {(tn, h): io_p.tile([PART, EC], F16, name=f"raw{tn}{h}",
                                       tag=f"raw{tn}{h}")
                    for tn in "kqv" for h in (0, 1)}
            tp = mid_p.tile([PART, W * EC], F16, name="tp", tag="tp")
            epr = mid_p.tile([PART, 2 * W * EC], BF16, name="epr", tag="epr")
            dn = mid_p.tile([PART, 2 * EC], BF16, name="dn", tag="dn")
            lnt = mid_p.tile([PART, GC], F32, name="lnt", tag="lnt")
            rr = mid_p.tile([PART, GC], BF16, name="rr", tag="rr")
            og = mid_p.tile([PART, EC], BF16, name="og", tag="og")

            qv, kv, vv = q_d.ap(), k_d.ap(), v_d.ap()
            ov = o_d.ap()

            H = EC // 2
            Q = EC // 4

            def load_q(tn, src, quarter, ring):
                """One DMA for both projections of one quarter into the
                raw tile for that half - plain HWDGE, one semaphore."""
                h = quarter // 2
                raw = raws[(tn, h)]
                rv = raw[:].rearrange("p (n c) -> p n c", n=NPROJ)
                qv_ = src.rearrange("n p (q c) -> p q n c", q=4)[:, quarter]
                ring.dma_start(rv[:, :, (quarter % 2) * Q:
                                  (quarter % 2 + 1) * Q], qv_)

            def load_qp(tn, src, quarter, proj, ring):
                """Single-projection quarter load - smallest first-wave
                unit, spread across both HWDGE rings."""
                h = quarter // 2
                raw = raws[(tn, h)]
                l0 = proj * H + (quarter % 2) * Q
                ring.dma_start(raw[:, l0:l0 + Q],
                               src[proj][:, quarter * Q:(quarter + 1) * Q])

            def load_h(tn, src, half, ring):
                raw = raws[(tn, half)]
                rv = raw[:].rearrange("p (n c) -> p n c", n=NPROJ)
                sv = src.rearrange("n p (h c) -> p h n c", h=2)[:, half]
                return ring.dma_start(rv, sv)

            def load_cce(dst, src, half, ring):
                """proj0 plain + proj1 DMA-accumulate (Q7 must be warm)."""
                c0, c1 = half * H, (half + 1) * H
                ring.dma_start(dst[:, c0:c1], src[0][:, c0:c1])
                nc.gpsimd.dma_start(dst[:, c0:c1], src[1][:, c0:c1],
                                    accum_op=ADD)

            def avg(tn, dst, c0, c1, eng=None):
                """dst[:, c0:c1] = proj0 + proj1 from the raw tile."""
                h = c0 // H
                raw = raws[(tn, h)]
                l0, l1 = c0 - h * H, c1 - h * H
                return (eng or nc.vector).tensor_add(
                    dst[:, c0:c1], raw[:, l0:l1], raw[:, H + l0:H + l1])

            # strided views
            tpv = tp[:].rearrange("p (w c) -> p w c", w=W)
            epv = epr[:].rearrange("p (k w c) -> p k w c", k=2, w=W)
            dnv = dn[:].rearrange("p (k c) -> p k c", k=2)

            def score(e_lo, e_hi):
                """t = Qs * Ks[route]."""
                for w, e0, j0, L in runs_for(e_lo, e_hi):
                    nc.vector.tensor_mul(
                        tp[:, w * EC + e0 * COLS: w * EC + (e0 + L) * COLS],
                        qs[:, e0 * COLS:(e0 + L) * COLS],
                        ks[:, j0 * COLS:(j0 + L) * COLS])

            def expprod(e_lo, e_hi):
                """e = exp(esc*gate*t), then prod = e * Vs[route].

                The beta gate rides the exp scale immediate: slot0 (self,
                gate 1) batches into one instruction per group; the other
                slots go one ACT instruction per (e,w) - ScalarE has the
                slack, and this keeps the gate multiply off the DVE."""
                c0, c1 = e_lo * COLS, e_hi * COLS
                nc.scalar.activation(epv[:, 0, 0, c0:c1], tp[:, c0:c1],
                                     EXPF, bias=0.0, scale=esc)
                for w in range(1, W):
                    for e in range(e_lo, e_hi):
                        sl = slice(w * EC + e * COLS, w * EC + (e + 1) * COLS)
                        nc.scalar.activation(
                            epr[:, sl], tp[:, sl], EXPF,
                            bias=0.0, scale=esc * float(gates_s[e, w]))
                for w, e0, j0, L in runs_for(e_lo, e_hi):
                    nc.vector.tensor_mul(
                        epr[:, (W + w) * EC + e0 * COLS:
                            (W + w) * EC + (e0 + L) * COLS],
                        epr[:, w * EC + e0 * COLS: w * EC + (e0 + L) * COLS],
                        vs[:, j0 * COLS:(j0 + L) * COLS])

            def finale(gc0, gc1, fin_chunks=1, store_rings=None):
                fc = (gc1 - gc0) // fin_chunks
                for f in range(fin_chunks):
                    c0, c1 = gc0 + f * fc, gc0 + (f + 1) * fc
                    l0, l1 = c0 - gc0, c1 - gc0
                    # den | num sums over w in two adds
                    nc.vector.tensor_add(dnv[:, :, c0:c1],
                                         epv[:, :, 0, c0:c1],
                                         epv[:, :, 1, c0:c1])
                    nc.vector.tensor_add(dnv[:, :, c0:c1], dnv[:, :, c0:c1],
                                         epv[:, :, 2, c0:c1])
                    # r = 0.5/den = exp(-ln(2*den)); 0.5 rides the ln scale
                    nc.scalar.activation(lnt[:, l0:l1], dn[:, c0:c1], LNF,
                                         bias=0.0, scale=2.0)
                    nc.scalar.activation(rr[:, l0:l1], lnt[:, l0:l1], EXPF,
                                         bias=0.0, scale=-1.0)
                    # out = num * r
                    nc.vector.tensor_mul(og[:, c0:c1],
                                         dn[:, EC + c0:EC + c1],
                                         rr[:, l0:l1])
                    if store_rings:
                        store_rings[f % len(store_rings)].dma_start(
                            ov[:, c0:c1], og[:, c0:c1])

            def store(c0, c1, ring=None):
                (ring or nc.sync).dma_start(ov[:, c0:c1], og[:, c0:c1])

            # All loads are plain HWDGE (no Q7 / CCE chains), 8 DMA ops
            # total so each gets its own completion-semaphore lane.  The
            # k/q quarters that unblock group 3 come first; h0 averaging
            # runs on the otherwise-idle GpSimd engine.
            # All loads plain HWDGE (CCE accumulate and GpSimd tensor ops
            # both measured as large critical-path losses).  q3 arrives as
            # per-projection 256KB ops striped over both rings; the four
            # big h-loads are gated behind the last quarter-average so
            # their completion receipts don't congest the critical
            # quarters' semaphores.
            load_qp("k", kv, 3, 0, nc.sync)
            load_qp("k", kv, 3, 1, nc.scalar)
            load_qp("q", qv, 3, 0, nc.sync)
            load_qp("q", qv, 3, 1, nc.scalar)
            load_q("k", kv, 2, nc.sync)
            load_q("q", qv, 2, nc.scalar)
            # pin the ACT table set that has BOTH exp and ln, after the
            # scalar ring's DMA issues so it doesn't delay them
            nc.scalar.add_instruction(mybir.InstLoadActFuncSet(
                name=nc.get_next_instruction_name(),
                act_func_set_id=ACT_SET_LN_EXP, ins=[], outs=[]))
            # averaging on DVE; q3 first so group 3's self-slot t can
            # issue after just two quarter adds
            gate_i = avg("k", ks, 3 * Q, 4 * Q)
            avg("q", qs, 3 * Q, 4 * Q)
            avg("k", ks, 2 * Q, 3 * Q)
            avg("q", qs, 2 * Q, 3 * Q)
            # The four big h-loads go out as plain SWDGE (Q7 is warm by
            # then, and HWDGE rings with >2 queued ops delay the earlier
            # ops' completion semaphores by several us).  Gated behind
            # the first quarter-average to keep the critical quarters'
            # data uncontended.
            for tn, src, hf, ring in (("v", vv, 1, nc.gpsimd),
                                      ("k", kv, 0, nc.gpsimd),
                                      ("q", qv, 0, nc.gpsimd),
                                      ("v", vv, 0, nc.gpsimd)):
                i0 = load_h(tn, src, hf, ring)
                tile.add_dep_helper(i0.ins, gate_i.ins, sync=True,
                                    reason="big-load wave gating")
            # group 3 (experts 12-15): fully inside h1
            score(12, 16)
            avg("v", vs, H, EC)    # after score so DVE never stalls on V
            avg("k", ks, 0, H)
            expprod(12, 16)
            avg("q", qs, 0, H)
            finale(3 * GC, 4 * GC)
            # group 2
            score(8, 12)
            avg("v", vs, 0, H)
            expprod(8, 12)
            finale(2 * GC, 3 * GC)
            store(2 * GC, 4 * GC)
            # group 0
            score(0, 4)
            expprod(0, 4)
            finale(0, GC, store_rings=[nc.scalar])
            # group 1 - last: fine-grained to shorten the tail
            score(4, 8)
            expprod(4, 6)
            expprod(6, 8)
            finale(GC, 2 * GC, fin_chunks=4,
                   store_rings=[nc.sync, nc.scalar])

    nc.compile()
    return nc


_cache: dict = {}


def _get_nc(routes_s: np.ndarray, gates_s: np.ndarray, esc: float):
    key = (routes_s.tobytes(), gates_s.tobytes(), float(esc))
    if key not in _cache:
        _cache[key] = _build_nc(routes_s, gates_s, esc)
    return _cache[key]


def _slot_sort(routes: np.ndarray, betas: np.ndarray):
    """Slot-permute so slot0 = self (gate 1); others sorted by offset."""
    gate = np.where(routes != np.arange(E, dtype=np.int32)[:, None],
                    1.0 / (1.0 + np.exp(-betas.astype(np.float64))),
                    1.0)
    routes_s = np.zeros((E, W), np.int32)
    gates_s = np.ones((E, W), np.float64)
    for e in range(E):
        slots = list(range(W))
        self_w = [w for w in slots if routes[e, w] == e]
        assert self_w, f"expert {e} missing self route"
        rest = [w for w in slots if w != self_w[0]]
        rest.sort(key=lambda w: int(routes[e, w]) - e)
        order = [self_w[0]] + rest
        routes_s[e] = routes[e, order]
        gates_s[e] = gate[e, order]
    return routes_s, gates_s.astype(np.float32)


def kernel(Q_proj, K_proj, V_proj, betas, temperature, routes, num_patches):
    Q = np.asarray(Q_proj, dtype=np.float32)
    K = np.asarray(K_proj, dtype=np.float32)
    V = np.asarray(V_proj, dtype=np.float32)
    betas = np.asarray(betas, dtype=np.float32)
    temp = np.asarray(temperature, dtype=np.float32)
    routes = np.asarray(routes, dtype=np.int32)
    assert int(num_patches) == E * P

    # Qs = Q0+Q1 (2x the mean); the 0.25 from both means is folded into
    # the exp scale esc together with sqrt(d)*|temperature|.
    esc = float(0.25 / (np.sqrt(np.float32(EXPERT_DIM)) * np.abs(temp[0])))
    routes_s, gates_s = _slot_sort(routes, betas)
    nc = _get_nc(routes_s, gates_s, esc)

    def prep(X):
        # [E, NPROJ, BS, P] -> [NPROJ, (b ph), (e c)] fp16
        return np.ascontiguousarray(
            X.reshape(E, NPROJ, BS, PH, COLS).transpose(1, 2, 3, 0, 4)
            .reshape(NPROJ, PART, EC).astype(np.float16))

    in_maps = []
    for c in range(NCORES):
        sl = slice(c * BS, (c + 1) * BS)
        in_maps.append({
            "q": prep(Q[:, :, sl, :]),
            "k": prep(K[:, :, sl, :]),
            "v": prep(V[:, :, sl, :]),
        })

    res = run_bass_kernel_spmd(nc, in_maps, list(range(NCORES)))
    out = np.empty((B, E * P), np.float32)
    for c in range(NCORES):
        o = np.asarray(res.results[c]["out"]).astype(np.float32)
        out[c * BS:(c + 1) * BS] = (
            o.reshape(BS, PH, E, COLS).transpose(0, 2, 1, 3)
            .reshape(BS, E * P))
    return out


# revision 38
# speedup vs baseline: 1.0428x; 1.0428x over previous
"""Cantor global attention kernel for Trainium2 (8 NeuronCores, SPMD).

Strategy: data-parallel over batch B=64 -> 8 cores x 8 rows each.
All device tensors are 16-bit: Q/K (and the pre-exp score t) in fp16
for exponent accuracy, everything after the exp in bf16 for range
safety (scores reach ~|20| so e^t needs bf16's exponent range).  The
host uploads inputs already transposed into the SBUF layout
[proj][128 part][e*256 col] so every DMA is a few large contiguous
descriptors, and converts the bf16 output back to f32.

Per core, partition = b*16 + p//256; each expert owns 256 columns;
the W=3 neighbor gather becomes column offsets baked from the runtime
routes (slot-permuted so slot0 = self).

Engine placement (per core):
  - projection averaging (Q,K,V): DMA-accumulate (CCE add) - free
  - t_w = Qs*Ks_route:  DVE tensor_mul fp16 (2x packed), run-batched
  - gate: t *= sigmoid(beta) per non-self (e,w): DVE tensor_scalar
    (4x mode), immediate baked from betas
  - e_w = exp(esc*t):   ScalarE activation, uniform scale immediate
    esc = 0.25/(sqrt(128)*|temp|)  ->  fully batched big instructions
  - prod_w = e_w*Vs:    DVE tensor_mul bf16, run-batched
  - den|num = sum_w:    2 DVE adds per group over a [k=2,w=3] strided
    view covering both reductions
  - r = 0.5/den:        ScalarE ln (fp32 out) then exp(-x+ln 0.5)
  - out = num*r:        DVE mul, stored bf16
"""

import math

import numpy as np

import concourse.bass as bass
import concourse.mybir as mybir
from concourse import bacc, tile
from concourse.bass_utils import run_bass_kernel_spmd

E, NPROJ, B, P = 16, 2, 64, 4096
W = 3
EXPERT_DIM = 128
NCORES = 8
BS = B // NCORES          # 8 batch rows per core
COLS = 256                # free-dim columns per expert slab
PH = P // COLS            # 16 partition sub-blocks per batch row
PART = BS * PH            # 128 SBUF partitions
EC = E * COLS             # 4096 cols per w-block
GROUP = 4                 # experts per compute group
NG = E // GROUP           # 4 groups
GC = GROUP * COLS         # 1024 cols per group
ACT_SET_LN_EXP = 6        # act_info.json natural_log_exp_and_others

F16 = mybir.dt.float16
BF16 = mybir.dt.bfloat16
F32 = mybir.dt.float32
EXPF = mybir.ActivationFunctionType.Exp
LNF = mybir.ActivationFunctionType.Ln
MULT = mybir.AluOpType.mult
ADD = mybir.AluOpType.add


def _runs(pairs):
    """Split [(e, j), ...] into maximal runs of consecutive e and j."""
    runs = []
    for e, j in pairs:
        if runs and runs[-1][0] + runs[-1][2] == e and runs[-1][1] + runs[-1][2] == j:
            runs[-1][2] += 1
        else:
            runs.append([e, j, 1])
    return runs


def _build_nc(routes_s: np.ndarray, gates_s: np.ndarray, esc: float):
    nc = bacc.Bacc("TRN2", target_bir_lowering=False, debug=False,
                   num_devices=NCORES)

    q_d = nc.dram_tensor("q", [NPROJ, PART, EC], F16, kind="ExternalInput")
    k_d = nc.dram_tensor("k", [NPROJ, PART, EC], F16, kind="ExternalInput")
    v_d = nc.dram_tensor("v", [NPROJ, PART, EC], F16, kind="ExternalInput")
    o_d = nc.dram_tensor("out", [PART, EC], BF16, kind="ExternalOutput")

    def runs_for(e_lo, e_hi):
        out = []
        for w in range(W):
            pairs = [(e, int(routes_s[e, w])) for e in range(e_lo, e_hi)]
            for e0, j0, L in _runs(pairs):
                out.append((w, e0, j0, L))
        return out

    with tile.TileContext(nc) as tc:
        with (
            tc.tile_pool(name="io", bufs=1) as io_p,
            tc.tile_pool(name="mid", bufs=1) as mid_p,
        ):
            qs = io_p.tile([PART, EC], F16, name="qs", tag="qs")
            ks = io_p.tile([PART, EC], F16, name="ks", tag="ks")
            vs = io_p.tile([PART, EC], F16, name="vs", tag="vs")
            raws = {(tn, h): io_p.tile([PART, EC], F16, name=f"raw{tn}{h}",
                                       tag=f"raw{tn}{h}")
                    for tn in "kqv" for h in (0, 1)}
            tp = mid_p.tile([PART, W * EC], F16, name="tp", tag="tp")
            epr = mid_p.tile([PART, 2 * W * EC], BF16, name="epr", tag="epr")
            dn = mid_p.tile([PART, 2 * EC], BF16, name="dn", tag="dn")
            lnt = mid_p.tile([PART, GC], F32, name="lnt", tag="lnt")
            rr = mid_p.tile([PART, GC], BF16, name="rr", tag="rr")
            og = mid_p.tile([PART, EC], BF16, name="og", tag="og")

            qv, kv, vv = q_d.ap(), k_d.ap(), v_d.ap()
            ov = o_d.ap()

            H = EC // 2
            Q = EC // 4

            def load_q(tn, src, quarter, ring):
                """One DMA for both projections of one quarter into the
                raw tile for that half - plain HWDGE, one semaphore."""
                h = quarter // 2
                raw = raws[(tn, h)]
                rv = raw[:].rearrange("p (n c) -> p n c", n=NPROJ)
                qv_ = src.rearrange("n p (q c) -> p q n c", q=4)[:, quarter]
                ring.dma_start(rv[:, :, (quarter % 2) * Q:
                                  (quarter % 2 + 1) * Q], qv_)

            def load_qp(tn, src, quarter, proj, ring):
                """Single-projection quarter load - smallest first-wave
                unit, spread across both HWDGE rings."""
                h = quarter // 2
                raw = raws[(tn, h)]
                l0 = proj * H + (quarter % 2) * Q
                ring.dma_start(raw[:, l0:l0 + Q],
                               src[proj][:, quarter * Q:(quarter + 1) * Q])

            def load_h(tn, src, half, ring):
                raw = raws[(tn, half)]
                rv = raw[:].rearrange("p (n c) -> p n c", n=NPROJ)
                sv = src.rearrange("n p (h c) -> p h n c", h=2)[:, half]
                return ring.dma_start(rv, sv)

            def load_cce(dst, src, half, ring):
                """proj0 plain + proj1 DMA-accumulate (Q7 must be warm)."""
                c0, c1 = half * H, (half + 1) * H
                ring.dma_start(dst[:, c0:c1], src[0][:, c0:c1])
                nc.gpsimd.dma_start(dst[:, c0:c1], src[1][:, c0:c1],
                                    accum_op=ADD)

            def avg(tn, dst, c0, c1, eng=None):
                """dst[:, c0:c1] = proj0 + proj1 from the raw tile."""
                h = c0 // H
                raw = raws[(tn, h)]
                l0, l1 = c0 - h * H, c1 - h * H
                return (eng or nc.vector).tensor_add(
                    dst[:, c0:c1], raw[:, l0:l1], raw[:, H + l0:H + l1])

            # strided views
            tpv = tp[:].rearrange("p (w c) -> p w c", w=W)
            epv = epr[:].rearrange("p (k w c) -> p k w c", k=2, w=W)
            dnv = dn[:].rearrange("p (k c) -> p k c", k=2)

            def score(e_lo, e_hi):
                """t = Qs * Ks[route]."""
                for w, e0, j0, L in runs_for(e_lo, e_hi):
                    nc.vector.tensor_mul(
                        tp[:, w * EC + e0 * COLS: w * EC + (e0 + L) * COLS],
                        qs[:, e0 * COLS:(e0 + L) * COLS],
                        ks[:, j0 * COLS:(j0 + L) * COLS])

            def expprod(e_lo, e_hi):
                """e = exp(esc*gate*t), then prod = e * Vs[route].

                The beta gate rides the exp scale immediate: slot0 (self,
                gate 1) batches into one instruction per group; the other
                slots go one ACT instruction per (e,w) - ScalarE has the
                slack, and this keeps the gate multiply off the DVE."""
                c0, c1 = e_lo * COLS, e_hi * COLS
                nc.scalar.activation(epv[:, 0, 0, c0:c1], tp[:, c0:c1],
                                     EXPF, bias=0.0, scale=esc)
                for w in range(1, W):
                    for e in range(e_lo, e_hi):
                        sl = slice(w * EC + e * COLS, w * EC + (e + 1) * COLS)
                        nc.scalar.activation(
                            epr[:, sl], tp[:, sl], EXPF,
                            bias=0.0, scale=esc * float(gates_s[e, w]))
                for w, e0, j0, L in runs_for(e_lo, e_hi):
                    nc.vector.tensor_mul(
                        epr[:, (W + w) * EC + e0 * COLS:
                            (W + w) * EC + (e0 + L) * COLS],
                        epr[:, w * EC + e0 * COLS: w * EC + (e0 + L) * COLS],
                        vs[:, j0 * COLS:(j0 + L) * COLS])

            def finale(gc0, gc1, fin_chunks=1, store_rings=None):
                fc = (gc1 - gc0) // fin_chunks
                for f in range(fin_chunks):
                    c0, c1 = gc0 + f * fc, gc0 + (f + 1) * fc
                    l0, l1 = c0 - gc0, c1 - gc0
                    # den | num sums over w in two adds
                    nc.vector.tensor_add(dnv[:, :, c0:c1],
                                         epv[:, :, 0, c0:c1],
                                         epv[:, :, 1, c0:c1])
                    nc.vector.tensor_add(dnv[:, :, c0:c1], dnv[:, :, c0:c1],
                                         epv[:, :, 2, c0:c1])
                    # r = 0.5/den = exp(-ln(2*den)); 0.5 rides the ln scale
                    nc.scalar.activation(lnt[:, l0:l1], dn[:, c0:c1], LNF,
                                         bias=0.0, scale=2.0)
                    nc.scalar.activation(rr[:, l0:l1], lnt[:, l0:l1], EXPF,
                                         bias=0.0, scale=-1.0)
                    # out = num * r
                    nc.vector.tensor_mul(og[:, c0:c1],
                                         dn[:, EC + c0:EC + c1],
                                         rr[:, l0:l1])
                    if store_rings:
                        store_rings[f % len(store_rings)].dma_start(
                            ov[:, c0:c1], og[:, c0:c1])

            def store(c0, c1, ring=None):
                (ring or nc.sync).dma_start(ov[:, c0:c1], og[:, c0:c1])

            # All loads are plain HWDGE (no Q7 / CCE chains), 8 DMA ops
            # total so each gets its own completion-semaphore lane.  The
            # k/q quarters that unblock group 3 come first; h0 averaging
            # runs on the otherwise-idle GpSimd engine.
            # All loads plain HWDGE (CCE accumulate and GpSimd tensor ops
            # both measured as large critical-path losses).  q3 arrives as
            # per-projection 256KB ops striped over both rings; the four
            # big h-loads are gated behind the last quarter-average so
            # their completion receipts don't congest the critical
            # quarters' semaphores.
            load_q("k", kv, 3, nc.sync)
            load_q("q", qv, 3, nc.scalar)
            load_q("k", kv, 2, nc.sync)
            load_q("q", qv, 2, nc.scalar)
            # pin the ACT table set that has BOTH exp and ln, after the
            # scalar ring's DMA issues so it doesn't delay them
            nc.scalar.add_instruction(mybir.InstLoadActFuncSet(
                name=nc.get_next_instruction_name(),
                act_func_set_id=ACT_SET_LN_EXP, ins=[], outs=[]))
            # averaging on DVE; q3 first so group 3's self-slot t can
            # issue after just two quarter adds
            gate_i = avg("k", ks, 3 * Q, 4 * Q)
            avg("q", qs, 3 * Q, 4 * Q)
            avg("k", ks, 2 * Q, 3 * Q)
            avg("q", qs, 2 * Q, 3 * Q)
            # The four big h-loads go out as plain SWDGE (Q7 is warm by
            # then, and HWDGE rings with >2 queued ops delay the earlier
            # ops' completion semaphores by several us).  Gated behind
            # the first quarter-average to keep the critical quarters'
            # data uncontended.
            for tn, src, hf, ring in (("v", vv, 1, nc.gpsimd),
                                      ("k", kv, 0, nc.gpsimd),
                                      ("q", qv, 0, nc.gpsimd),
                                      ("v", vv, 0, nc.gpsimd)):
                i0 = load_h(tn, src, hf, ring)
                tile.add_dep_helper(i0.ins, gate_i.ins, sync=True,
                                    reason="big-load wave gating")
            # group 3 (experts 12-15): fully inside h1
            score(12, 16)
            avg("v", vs, H, EC)    # after score so DVE never stalls on V
            avg("k", ks, 0, H)
            expprod(12, 16)
            avg("q", qs, 0, H)
            finale(3 * GC, 4 * GC)
            # group 2
            score(8, 12)
            avg("v", vs, 0, H)
            expprod(8, 12)
            finale(2 * GC, 3 * GC)
            store(2 * GC, 4 * GC)
            # group 0
            score(0, 4)
            expprod(0, 4)
            finale(0, GC, store_rings=[nc.scalar])
            # group 1 - last: fine-grained to shorten the tail
            score(4, 8)
            expprod(4, 6)
            expprod(6, 8)
            finale(GC, 2 * GC, fin_chunks=4,
                   store_rings=[nc.sync, nc.scalar])

    nc.compile()
    return nc


_cache: dict = {}


def _get_nc(routes_s: np.ndarray, gates_s: np.ndarray, esc: float):
    key = (routes_s.tobytes(), gates_s.tobytes(), float(esc))
    if key not in _cache:
        _cache[key] = _build_nc(routes_s, gates_s, esc)
    return _cache[key]


def _slot_sort(routes: np.ndarray, betas: np.ndarray):
    """Slot-permute so slot0 = self (gate 1); others sorted by offset."""
    gate = np.where(routes != np.arange(E, dtype=np.int32)[:, None],
                    1.0 / (1.0 + np.exp(-betas.astype(np.float64))),
                    1.0)
    routes_s = np.zeros((E, W), np.int32)
    gates_s = np.ones((E, W), np.float64)
    for e in range(E):
        slots = list(range(W))
        self_w = [w for w in slots if routes[e, w] == e]
        assert self_w, f"expert {e} missing self route"
        rest = [w for w in slots if w != self_w[0]]
        rest.sort(key=lambda w: int(routes[e, w]) - e)
        order = [self_w[0]] + rest
        routes_s[e] = routes[e, order]
        gates_s[e] = gate[e, order]
    return routes_s, gates_s.astype(np.float32)


def kernel(Q_proj, K_proj, V_proj, betas, temperature, routes, num_patches):
    Q = np.asarray(Q_proj, dtype=np.float32)
    K = np.asarray(K_proj, dtype=np.float32)
    V = np.asarray(V_proj, dtype=np.float32)
    betas = np.asarray(betas, dtype=np.float32)
    temp = np.asarray(temperature, dtype=np.float32)
    routes = np.asarray(routes, dtype=np.int32)
    assert int(num_patches) == E * P

    # Qs = Q0+Q1 (2x the mean); the 0.25 from both means is folded into
    # the exp scale esc together with sqrt(d)*|temperature|.
    esc = float(0.25 / (np.sqrt(np.float32(EXPERT_DIM)) * np.abs(temp[0])))
    routes_s, gates_s = _slot_sort(routes, betas)
    nc = _get_nc(routes_s, gates_s, esc)

    def prep(X):
        # [E, NPROJ, BS, P] -> [NPROJ, (b ph), (e c)] fp16
        return np.ascontiguousarray(
            X.reshape(E, NPROJ, BS, PH, COLS).transpose(1, 2, 3, 0, 4)
            .reshape(NPROJ, PART, EC).astype(np.float16))

    in_maps = []
    for c in range(NCORES):
        sl = slice(c * BS, (c + 1) * BS)
        in_maps.append({
            "q": prep(Q[:, :, sl, :]),
            "k": prep(K[:, :, sl, :]),
            "v": prep(V[:, :, sl, :]),
        })

    res = run_bass_kernel_spmd(nc, in_maps, list(range(NCORES)))
    out = np.empty((B, E * P), np.float32)
    for c in range(NCORES):
        o = np.asarray(res.results[c]["out"]).astype(np.float32)
        out[c * BS:(c + 1) * BS] = (
            o.reshape(BS, PH, E, COLS).transpose(0, 2, 1, 3)
            .reshape(BS, E * P))
    return out
